# revision 24
# baseline (speedup 1.0000x reference)
"""Trainium2 Bass kernel for the quantized LM-head (nn_LmHeadTender).

Math (per core, vocab-sharded, 4000 vocab rows/core):
  reference computes
    Wl   = dequant_int4(lm_weight)            # per-row scale sw = rowmax/7
    y    = dequant_int4(x, per-(chunk,channel) scale s = tmax*2^(bucket-13)/7)
    out  = y @ Wl.T
  Every scale is factored out of the matmul so both matmul operands are
  small integers (times powers of two) EXACTLY representable in fp8-e5m2
  (3 significand bits cover ints -8..7; exponent covers 2^-13..2^3):
    qw  in [-8, 7]                 (weight int values)
    yq  = qx * 2^(bucket-13)       (activation ints scaled by a power of 2)
    out[t, v] = (tmax_c/7) * sw[v] * sum_h yq[t, h] * qw[v, h]
  The fp8 matmul runs in DoubleRow perf mode (256-deep contraction per
  instruction, ~2x bf16 ALU throughput) and accumulates exactly in fp32
  PSUM. The device applies only the per-chunk tmax_c/7 scale; the
  per-vocab-row sw[v] (= max(rowmax/7, 1e-9), recomputable bitwise on the
  host from lm_weight alone) is applied by the host after the gather,
  which removes the on-device sw broadcast entirely.

  The reference clip to [-8,7] is dead code: |x/s| <= 7 and |w/sw| <= 7
  by construction of the scales, so quantization needs only the
  round-to-nearest-even magic-number trick.

Pipeline: weight tiles, activation chunks, and matmul token-groups are
interleaved in issue order so the PE matmul stream starts ~30us in and
everything else (DMA, DVE, ACT, GPSIMD) hides under it:
  - weight tile: DMA w f32, abs-rowmax (GPSIMD), 1/sw (DVE), quantize
    (ACT round pass + DVE/ACT sub pass) -> bf16 ints in DRAM qw_d.
  - act chunk: DMA x f32, PE-transpose (4 tiles per PSUM bank), partial
    per-channel abs-max directly on PSUM (DVE, pipelines with the
    copies), chunk stats (tmax/bucket/2^b via exact IEEE bit tricks),
    2-pass quantization alternated across DVE/ACT -> fp8e5 y in DRAM
    y_d [128, 32, T] (p-major: single 3D-AP DMAs).
  - mm: vocab in 4 sweeps of 2x512-col blocks (last 512+416); per sweep
    the qw block is DMA-transposed (bf16) + cast to fp8e5 once into a
    2-deep SBUF ring and reused by all 32 token blocks; stationary y tile
    shared by the 2 blocks; PSUM accumulates 16 k-pairs; epilogue just
    scales by tmax_c/7 (per-partition) -> bf16 out.
"""

import numpy as np
from collections import deque
from contextlib import ExitStack

import concourse.bass as bass
import concourse.tile as tile
from concourse import bacc, masks, mybir
from concourse.bass_isa import ReduceOp
from concourse.bass_utils import run_bass_kernel_spmd

FP = mybir.dt.float32
BF = mybir.dt.bfloat16
F8 = mybir.dt.float8e5
I32 = mybir.dt.int32
ALU = mybir.AluOpType
AX = mybir.AxisListType
AF = mybir.ActivationFunctionType
DR = mybir.MatmulPerfMode.DoubleRow

T = 4096            # tokens (2*2048)
H = 4096            # hidden
V = 32000           # vocab
NCORE = 8
VSH = V // NCORE    # 4000 vocab rows per core
CHUNK = 256
NCHUNK = T // CHUNK  # 16
DECOMP = 14
QMAX = 7.0
MAGIC = 12582912.0   # 1.5 * 2^23: round-to-nearest-even via add/sub
C7 = float(np.float32(1.0) / np.float32(7.0))  # fl(1/7); no divide op

KT = H // 128        # 32 k tiles
VT = 32              # weight row tiles (31 full + 1 of 32 rows)
KP = KT // 2         # 16 k pairs for DoubleRow
TB = T // 128        # 32 token blocks

VB_OFF = [0, 512, 1024, 1536, 2048, 2560, 3072, 3584]
VB_W = [512] * 7 + [416]
SWEEPS = [(0, 1), (2, 3), (4, 5), (6, 7)]


def _vt_rows(m):
    r0 = m * 128
    return r0, min(r0 + 128, VSH) - r0


class _Emitter:
    def __init__(self, ctx, tc, x_d, w_d, out_d):
        self.tc = tc
        self.nc = tc.nc
        self.x_d = x_d
        self.w_d = w_d
        self.out_d = out_d
        nc = self.nc

        cpool = ctx.enter_context(tc.tile_pool(name="consts", bufs=1))
        self.ident = cpool.tile([128, 128], FP)
        masks.make_identity(nc, self.ident[:])
        self.magic_col = cpool.tile([128, 1], FP)
        nc.vector.memset(self.magic_col[:], MAGIC)
        self.sw_pk = cpool.tile([128, 32], FP)    # sw packed; v = m*128+p
        nc.vector.memset(self.sw_pk[:], 0.0)
        self.m7_all = cpool.tile([128, 16], FP)   # tmax_c/7 per chunk

        dpool = ctx.enter_context(tc.tile_pool(name="dram", bufs=1,
                                               space="DRAM"))
        self.qw_d = dpool.tile([VSH, H], BF)      # weight ints, [v, h]
        self.y_d = dpool.tile([128, NCHUNK, KT, CHUNK], F8)  # chunk-major

        self.mps_pool = ctx.enter_context(
            tc.tile_pool(name="mps", bufs=6, space="PSUM"))
        self.stg_pool = ctx.enter_context(tc.tile_pool(name="stg", bufs=2))
        self.ygrp_pool = ctx.enter_context(tc.tile_pool(name="ygrp", bufs=2))
        self.land_pool = ctx.enter_context(tc.tile_pool(name="land", bufs=2))
        self.qw_pool = ctx.enter_context(tc.tile_pool(name="qw", bufs=2))
        self._alt = 0
        self._alt2 = 0
        self._alt3 = 0
        self.qw_sw = {}

    def _copy(self, dst, src):
        self._alt ^= 1
        if self._alt:
            self.nc.vector.tensor_copy(dst, src)
        else:
            self.nc.scalar.copy(dst, src)

    def _copy3(self, dst, src):
        self._alt3 = (self._alt3 + 1) % 3
        if self._alt3 == 0:
            self.nc.gpsimd.tensor_copy(dst, src)
        elif self._alt3 == 1:
            self.nc.vector.tensor_copy(dst, src)
        else:
            self.nc.scalar.copy(dst, src)

    # ---------------- weight phase ----------------
    def weight_tile(self, m, wq_pool, ws_pool, qst_pool):
        nc = self.nc
        r0, nr = _vt_rows(m)
        halves = []
        for hh in range(2):
            w_nat = wq_pool.tile([128, 2048], FP, tag="w_nat")
            nc.sync.dma_start(
                w_nat[:nr], self.w_d[r0:r0 + nr, hh * 2048:(hh + 1) * 2048])
            halves.append(w_nat)
        rmax = ws_pool.tile([128, 2], FP, tag="rmax")
        for hh in range(2):
            nc.vector.tensor_reduce(
                rmax[:nr, hh:hh + 1], halves[hh][:nr], axis=AX.X, op=ALU.max,
                apply_absolute_value=True)
        rall = ws_pool.tile([128, 1], FP, tag="rall")
        nc.vector.tensor_reduce(
            rall[:nr], rmax[:nr], axis=AX.X, op=ALU.max)
        # sw = max(rmax/7, 1e-9)
        nc.vector.tensor_scalar(
            self.sw_pk[:nr, m:m + 1], rall[:nr], C7, 1e-9, ALU.mult, ALU.max)
        rw = ws_pool.tile([128, 1], FP, tag="rw")
        nc.vector.reciprocal(rw[:nr], self.sw_pk[:nr, m:m + 1])
        for hh in range(2):
            qst = qst_pool.tile([128, 2048], BF, tag="qst")
            nc.scalar.activation(
                halves[hh][:nr], halves[hh][:nr], AF.Identity,
                bias=self.magic_col[:nr], scale=rw[:nr])
            nc.scalar.activation(
                qst[:nr], halves[hh][:nr], AF.Identity,
                bias=self.neg_magic_col[:nr], scale=1.0)
            nc.sync.dma_start(
                self.qw_d[r0:r0 + nr, hh * 2048:(hh + 1) * 2048], qst[:nr])

    # ---------------- activation phase ----------------
    def act_chunk(self, c, xin_pool, xT_pool, st_pool, y_pool, xps_pool):
        nc = self.nc
        xT_lo = xT_pool.tile([128, KT // 2, CHUNK], FP, tag="xT")
        xT_hi = xT_pool.tile([128, KT // 2, CHUNK], FP, tag="xT")
        xT = [xT_lo, xT_hi]
        pcm = st_pool.tile([128, 2, KT], FP, tag="pcm")
        for th in range(2):
            xins = []
            for hh in range(2):
                xnat = xin_pool.tile([128, 2048], FP, tag="xn")
                nc.sync.dma_start(
                    xnat[:],
                    self.x_d[c * CHUNK + th * 128:c * CHUNK + (th + 1) * 128,
                             hh * 2048:(hh + 1) * 2048])
                xins.append(xnat)
            for kq in range(KT // 4):
                pst = xps_pool.tile([128, 512], FP, tag="pst")
                for j in range(4):
                    k = kq * 4 + j
                    nc.tensor.transpose(
                        pst[:, j * 128:(j + 1) * 128],
                        xins[k // 16][:, (k % 16) * 128:(k % 16 + 1) * 128],
                        self.ident[:])
                p3 = pst[:].rearrange("p (k t) -> p k t", k=4)
                nc.scalar.copy(
                    xT[kq // 4][:, (kq % 4) * 4:(kq % 4) * 4 + 4,
                                th * 128:(th + 1) * 128], p3)
                nc.vector.tensor_reduce(
                    pcm[:, th, kq * 4:kq * 4 + 4], p3, axis=AX.X, op=ALU.max,
                    apply_absolute_value=True)
        # ---- stats ----
        cmax = st_pool.tile([128, KT], FP, tag="cmax")
        nc.vector.scalar_tensor_tensor(
            cmax[:], pcm[:, 0, :], 0.0, pcm[:, 1, :],
            op0=ALU.bypass, op1=ALU.max)
        cmx1 = st_pool.tile([128, 1], FP, tag="cmx1")
        nc.vector.tensor_reduce(
            cmx1[:], cmax[:], axis=AX.X, op=ALU.max)
        tmax_b = st_pool.tile([128, 1], FP, tag="tmax_b")
        nc.gpsimd.partition_all_reduce(
            tmax_b[:], cmx1[:], 128, ReduceOp.max)
        nc.vector.tensor_scalar(
            self.m7_all[:, c:c + 1], tmax_b[:], C7, None, ALU.mult)
        # ---- pw = clamp(2^ceil(log2(cmax/tmax)), 2^-13, 1), exactly:
        # smallest power of two >= u via IEEE bit surgery ----
        rtm = st_pool.tile([128, 1], FP, tag="rtm")
        nc.vector.reciprocal(rtm[:], tmax_b[:])
        u = st_pool.tile([128, KT], FP, tag="u")
        nc.vector.tensor_scalar(u[:], cmax[:], rtm[:], None, ALU.mult)
        ui = u[:].bitcast(I32)
        tm = st_pool.tile([128, KT], I32, tag="tm")
        nc.vector.tensor_scalar(tm[:], ui, 0x007fffff, None, ALU.bitwise_and)
        # m + 0x7fffff has bit 23 set iff mantissa m != 0
        nc.vector.tensor_scalar(tm[:], tm[:], 0x007fffff, None, ALU.add)
        nc.vector.tensor_scalar(tm[:], tm[:], 0x00800000, None,
                                ALU.bitwise_and)
        te = st_pool.tile([128, KT], I32, tag="te")
        nc.vector.tensor_scalar(te[:], ui, 0x7f800000, None, ALU.bitwise_and)
        pw_t = st_pool.tile([128, KT], I32, tag="pw_t")
        nc.vector.scalar_tensor_tensor(
            pw_t[:], te[:], 0, tm[:], op0=ALU.bypass, op1=ALU.add)
        pw_f = st_pool.tile([128, KT], FP, tag="pw_f")
        nc.vector.tensor_scalar(
            pw_f[:], pw_t[:].bitcast(FP), 1.0, 2.0 ** -13, ALU.min, ALU.max)
        pw = pw_f[:]
        # ---- r = 1/max(tmax*pw/7, 1e-9); mgpw = -MAGIC*pw ----
        ch_thr = st_pool.tile([128, KT], FP, tag="ch_thr")
        nc.vector.tensor_scalar(
            ch_thr[:], pw, tmax_b[:], None, ALU.mult)
        s_t = st_pool.tile([128, KT], FP, tag="s_t")
        nc.vector.tensor_scalar(
            s_t[:], ch_thr[:], C7, 1e-9, ALU.mult, ALU.max)
        r_t = st_pool.tile([128, KT], FP, tag="r_t")
        nc.vector.reciprocal(r_t[:], s_t[:])
        # ---- quantize: y = round(x*r) * pw  (fp8e5, exact) ----
        if c % 2 == 0:
            self.y_st = y_pool.tile([128, 2, KT, CHUNK], F8, tag="y_st")
        for k in range(KT):
            sl = xT[k // 16][:, k % 16, :]
            r_col = r_t[:, k:k + 1]
            pw_col = pw_f[:, k:k + 1]
            dst = self.y_st[:, c % 2, k, :]
            self._alt2 ^= 1
            if self._alt2:
                nc.scalar.activation(
                    sl, sl, AF.Identity, bias=self.magic_col[:], scale=r_col)
            else:
                nc.vector.tensor_scalar(
                    sl, sl, r_col, MAGIC, ALU.mult, ALU.add)
            nc.vector.tensor_scalar(
                dst, sl, MAGIC, pw_col, ALU.subtract, ALU.mult)
        if c % 2 == 1:
            nc.sync.dma_start(
                self.y_d[:, c - 1:c + 1, :, :], self.y_st[:])

    # ------------- qw sweep load (transpose + fp8 cast), spreadable ------
    def load_qw_part(self, s, k_lo, k_hi):
        nc = self.nc
        if s not in self.qw_sw:
            qw_sw = self.qw_pool.tile([128, KT, 1024], F8, tag="qw")
            self.qw_sw[s] = qw_sw
        qw_sw = self.qw_sw[s]
        for k in range(k_lo, k_hi):
            loc = 0
            for b in SWEEPS[s]:
                vo, w = VB_OFF[b], VB_W[b]
                land = self.land_pool.tile([128, 512], BF, tag="land")
                nc.sync.dma_start(
                    land[:, :w],
                    self.qw_d[vo:vo + w, k * 128:(k + 1) * 128],
                    transpose=True)
                self._copy(qw_sw[:, k, loc:loc + w], land[:, :w])
                loc += w

    # ---------------- matmul: one 256-token group of one sweep ----------
    def mm_grp(self, s, gc):
        nc = self.nc
        qw_sw = self.qw_sw[s]
        ba, bb = SWEEPS[s]
        wa, wb = VB_W[ba], VB_W[bb]
        vo = VB_OFF[ba]
        ygrp = self.ygrp_pool.tile([128, KT, CHUNK], F8, tag="yg")
        nc.sync.dma_start(ygrp[:], self.y_d[:, gc, :, :])
        m7c = self.m7_all[:, gc:gc + 1]
        for ti in range(2):
            tb = gc * 2 + ti
            ps_a = self.mps_pool.tile([128, 512], FP, tag="ps")
            ps_b = self.mps_pool.tile([128, 512], FP, tag="ps")
            for kp in range(KP):
                lhs = ygrp[:, 2 * kp:2 * kp + 2, ti * 128:(ti + 1) * 128]
                nc.tensor.matmul(
                    ps_a[:, :wa], lhs, qw_sw[:, 2 * kp:2 * kp + 2, 0:wa],
                    start=(kp == 0), stop=(kp == KP - 1), perf_mode=DR)
                nc.tensor.matmul(
                    ps_b[:, :wb], lhs,
                    qw_sw[:, 2 * kp:2 * kp + 2, 512:512 + wb],
                    start=(kp == 0), stop=(kp == KP - 1), perf_mode=DR)
            stg = self.stg_pool.tile([128, 1024], BF, tag="stg")
            self._alt2 ^= 1
            if self._alt2:
                nc.vector.tensor_scalar(
                    stg[:, 0:wa], ps_a[:, :wa], m7c, None, ALU.mult)
                nc.scalar.activation(
                    stg[:, wa:wa + wb], ps_b[:, :wb], AF.Copy, scale=m7c)
            else:
                nc.scalar.activation(
                    stg[:, 0:wa], ps_a[:, :wa], AF.Copy, scale=m7c)
                nc.vector.tensor_scalar(
                    stg[:, wa:wa + wb], ps_b[:, :wb], m7c, None, ALU.mult)
            nc.sync.dma_start(
                self.out_d[tb * 128:(tb + 1) * 128, vo:vo + wa + wb],
                stg[:, :wa + wb])


def _emit(ctx, tc, x_d, w_d, out_d):
    em = _Emitter(ctx, tc, x_d, w_d, out_d)
    nc = em.nc

    with ExitStack() as stage_a:
        cp2 = stage_a.enter_context(tc.tile_pool(name="consts2", bufs=1))
        em.neg_magic_col = cp2.tile([128, 1], FP)
        nc.vector.memset(em.neg_magic_col[:], -MAGIC)
        wq_pool = stage_a.enter_context(tc.tile_pool(name="wq", bufs=3))
        ws_pool = stage_a.enter_context(tc.tile_pool(name="wsm", bufs=2))
        qst_pool = stage_a.enter_context(tc.tile_pool(name="qst", bufs=1))
        xin_pool = stage_a.enter_context(tc.tile_pool(name="xin", bufs=3))
        xT_pool = stage_a.enter_context(tc.tile_pool(name="xT", bufs=3))
        st_pool = stage_a.enter_context(tc.tile_pool(name="xst", bufs=1))
        y_pool = stage_a.enter_context(tc.tile_pool(name="yst", bufs=1))
        xps_pool = stage_a.enter_context(
            tc.tile_pool(name="xps", bufs=2, space="PSUM"))

        # mm groups become issueable as their y chunk + qw sweep land.
        pending = deque()
        issued_chunks = [False] * NCHUNK
        loaded = [False] * 4

        def make_ready(c=None, s=None):
            if c is not None:
                issued_chunks[c] = True
            if s is not None:
                loaded[s] = True
            for sw in range(4):
                if loaded[sw]:
                    for gc in range(NCHUNK):
                        if issued_chunks[gc] and (sw, gc) not in scheduled:
                            scheduled.add((sw, gc))
                            pending.append((sw, gc))

        scheduled = set()

        def drain(n):
            for _ in range(min(n, len(pending))):
                s, gc = pending.popleft()
                em.mm_grp(s, gc)

        for i in range(32):
            em.weight_tile(i, wq_pool, ws_pool, qst_pool)
            if 8 <= i < 12:
                em.load_qw_part(0, (i - 8) * 8, (i - 7) * 8)
                if i == 11:
                    make_ready(s=0)
            if 16 <= i < 20:
                em.load_qw_part(1, (i - 16) * 8, (i - 15) * 8)
                if i == 19:
                    make_ready(s=1)
            if i % 2 == 1:
                c = (i - 1) // 2
                em.act_chunk(c, xin_pool, xT_pool, st_pool, y_pool, xps_pool)
                if c >= 1:
                    make_ready(c=c - 1)
                drain(2)
        make_ready(c=NCHUNK - 1)
        drain(len(pending))

    # sweeps 2 and 3: qw ring slots reused after sweeps 0/1 mm finished;
    # sweep 3's load is spread under sweep 2's matmuls
    em.load_qw_part(2, 0, KT)
    for s in (2, 3):
        for gc in range(NCHUNK):
            em.mm_grp(s, gc)
            if s == 2 and 4 <= gc < 12:
                em.load_qw_part(3, (gc - 4) * 4, (gc - 3) * 4)


_CACHED = None


def _build():
    global _CACHED
    if _CACHED is not None:
        return _CACHED
    nc = bacc.Bacc(
        "TRN2", target_bir_lowering=False, debug=False,
        enable_asserts=False, num_devices=NCORE)
    x_d = nc.dram_tensor("x", (T, H), FP, kind="ExternalInput").ap()
    w_d = nc.dram_tensor("w", (VSH, H), FP, kind="ExternalInput").ap()
    out_d = nc.dram_tensor("out", (T, VSH), BF, kind="ExternalOutput").ap()
    with tile.TileContext(nc) as tc:
        with ExitStack() as ctx:
            _emit(ctx, tc, x_d, w_d, out_d)
    nc.compile()
    _CACHED = nc
    return nc


def kernel(hidden_states: np.ndarray, lm_weight: np.ndarray) -> np.ndarray:
    b, t, h = hidden_states.shape
    assert (b * t, h) == (T, H) and lm_weight.shape == (V, H)
    x_full = np.ascontiguousarray(
        hidden_states.reshape(T, H).astype(np.float32))
    w_full = np.ascontiguousarray(lm_weight.astype(np.float32))
    in_maps = [{"x": x_full, "w": w_full[c * VSH:(c + 1) * VSH]}
               for c in range(NCORE)]
    nc = _build()
    res = run_bass_kernel_spmd(nc, in_maps, core_ids=list(range(NCORE)))
    # device output lacks the per-vocab-row sw scale; recompute it here
    # exactly as the device would (f32 rowmax * fl(1/7), floored at 1e-9)
    rowmax = np.abs(w_full).max(axis=1)
    sw = np.maximum(rowmax * np.float32(1.0 / 7.0), np.float32(1e-9))
    full = np.empty((T, V), dtype=np.float32)
    for c in range(NCORE):
        sl = slice(c * VSH, (c + 1) * VSH)
        full[:, sl] = np.asarray(res.results[c]["out"]).astype(np.float32)
        full[:, sl] *= sw[sl][None, :]
    return full.reshape(b, t, V)


# revision 25
# speedup vs baseline: 1.0734x; 1.0734x over previous
"""Trainium2 Bass kernel for the quantized LM-head (nn_LmHeadTender).

Math (per core, vocab-sharded, 4000 vocab rows/core):
  reference computes
    Wl   = dequant_int4(lm_weight)            # per-row scale sw = rowmax/7
    y    = dequant_int4(x, per-(chunk,channel) scale s = tmax*2^(bucket-13)/7)
    out  = y @ Wl.T
  Every scale is factored out of the matmul so both matmul operands are
  small integers (times powers of two) EXACTLY representable in fp8-e5m2
  (3 significand bits cover ints -8..7; exponent covers 2^-13..2^3):
    qw  in [-8, 7]                 (weight int values)
    yq  = qx * 2^(bucket-13)       (activation ints scaled by a power of 2)
    out[t, v] = (tmax_c/7) * sw[v] * sum_h yq[t, h] * qw[v, h]
  The fp8 matmul runs in DoubleRow perf mode (256-deep contraction per
  instruction, ~2x bf16 ALU throughput) and accumulates exactly in fp32
  PSUM. The device applies only the per-chunk tmax_c/7 scale; the
  per-vocab-row sw[v] (= max(rowmax/7, 1e-9), recomputable bitwise on the
  host from lm_weight alone) is applied by the host after the gather,
  which removes the on-device sw broadcast entirely.

  The reference clip to [-8,7] is dead code: |x/s| <= 7 and |w/sw| <= 7
  by construction of the scales, so quantization needs only the
  round-to-nearest-even magic-number trick.

Pipeline: weight tiles, activation chunks, and matmul token-groups are
interleaved in issue order so the PE matmul stream starts ~30us in and
everything else (DMA, DVE, ACT, GPSIMD) hides under it:
  - weight tile: DMA w f32, abs-rowmax (GPSIMD), 1/sw (DVE), quantize
    (ACT round pass + DVE/ACT sub pass) -> bf16 ints in DRAM qw_d.
  - act chunk: DMA x f32, PE-transpose (4 tiles per PSUM bank), partial
    per-channel abs-max directly on PSUM (DVE, pipelines with the
    copies), chunk stats (tmax/bucket/2^b via exact IEEE bit tricks),
    2-pass quantization alternated across DVE/ACT -> fp8e5 y in DRAM
    y_d [128, 32, T] (p-major: single 3D-AP DMAs).
  - mm: vocab in 4 sweeps of 2x512-col blocks (last 512+416); per sweep
    the qw block is DMA-transposed (bf16) + cast to fp8e5 once into a
    2-deep SBUF ring and reused by all 32 token blocks; stationary y tile
    shared by the 2 blocks; PSUM accumulates 16 k-pairs; epilogue just
    scales by tmax_c/7 (per-partition) -> bf16 out.
"""

import numpy as np
from collections import deque
from contextlib import ExitStack

import concourse.bass as bass
import concourse.tile as tile
from concourse import bacc, masks, mybir
from concourse.bass_isa import ReduceOp
from concourse.bass_utils import run_bass_kernel_spmd

FP = mybir.dt.float32
BF = mybir.dt.bfloat16
F8 = mybir.dt.float8e5
I32 = mybir.dt.int32
ALU = mybir.AluOpType
AX = mybir.AxisListType
AF = mybir.ActivationFunctionType
DR = mybir.MatmulPerfMode.DoubleRow

T = 4096            # tokens (2*2048)
H = 4096            # hidden
V = 32000           # vocab
NCORE = 8
VSH = V // NCORE    # 4000 vocab rows per core
CHUNK = 256
NCHUNK = T // CHUNK  # 16
DECOMP = 14
QMAX = 7.0
MAGIC = 12582912.0   # 1.5 * 2^23: round-to-nearest-even via add/sub
C7 = float(np.float32(1.0) / np.float32(7.0))  # fl(1/7); no divide op

KT = H // 128        # 32 k tiles
VT = 32              # weight row tiles (31 full + 1 of 32 rows)
KP = KT // 2         # 16 k pairs for DoubleRow
TB = T // 128        # 32 token blocks

VB_OFF = [0, 512, 1024, 1536, 2048, 2560, 3072, 3584]
VB_W = [512] * 7 + [416]
SWEEPS = [(0, 1), (2, 3), (4, 5), (6, 7)]


def _vt_rows(m):
    r0 = m * 128
    return r0, min(r0 + 128, VSH) - r0


class _Emitter:
    def __init__(self, ctx, tc, x_d, w_d, out_d):
        self.tc = tc
        self.nc = tc.nc
        self.x_d = x_d
        self.w_d = w_d
        self.out_d = out_d
        nc = self.nc

        cpool = ctx.enter_context(tc.tile_pool(name="consts", bufs=1))
        self.ident = cpool.tile([128, 128], FP)
        masks.make_identity(nc, self.ident[:])
        self.magic_col = cpool.tile([128, 1], FP)
        nc.vector.memset(self.magic_col[:], MAGIC)
        self.sw_pk = cpool.tile([128, 32], FP)    # sw packed; v = m*128+p
        nc.vector.memset(self.sw_pk[:], 0.0)
        self.m7_all = cpool.tile([128, 16], FP)   # tmax_c/7 per chunk

        dpool = ctx.enter_context(tc.tile_pool(name="dram", bufs=1,
                                               space="DRAM"))
        self.qw_d = dpool.tile([VSH, H], BF)      # weight ints, [v, h]
        self.y_d = dpool.tile([128, NCHUNK, KT, CHUNK], F8)  # chunk-major

        self.mps_pool = ctx.enter_context(
            tc.tile_pool(name="mps", bufs=6, space="PSUM"))
        self.stg_pool = ctx.enter_context(tc.tile_pool(name="stg", bufs=2))
        self.ygrp_pool = ctx.enter_context(tc.tile_pool(name="ygrp", bufs=2))
        self.land_pool = ctx.enter_context(tc.tile_pool(name="land", bufs=2))
        self.qw_pool = ctx.enter_context(tc.tile_pool(name="qw", bufs=2))
        self._alt = 0
        self._alt2 = 0
        self._alt3 = 0
        self.qw_sw = {}

    def _copy(self, dst, src):
        self._alt ^= 1
        if self._alt:
            self.nc.vector.tensor_copy(dst, src)
        else:
            self.nc.scalar.copy(dst, src)

    def _copy3(self, dst, src):
        self._alt3 = (self._alt3 + 1) % 3
        if self._alt3 == 0:
            self.nc.gpsimd.tensor_copy(dst, src)
        elif self._alt3 == 1:
            self.nc.vector.tensor_copy(dst, src)
        else:
            self.nc.scalar.copy(dst, src)

    # ---------------- weight phase ----------------
    def weight_tile(self, m, wq_pool, ws_pool, qst_pool):
        nc = self.nc
        r0, nr = _vt_rows(m)
        halves = []
        for hh in range(2):
            w_nat = wq_pool.tile([128, 2048], FP, tag="w_nat")
            nc.sync.dma_start(
                w_nat[:nr], self.w_d[r0:r0 + nr, hh * 2048:(hh + 1) * 2048])
            halves.append(w_nat)
        rmax = ws_pool.tile([128, 2], FP, tag="rmax")
        for hh in range(2):
            nc.vector.tensor_reduce(
                rmax[:nr, hh:hh + 1], halves[hh][:nr], axis=AX.X, op=ALU.max,
                apply_absolute_value=True)
        rall = ws_pool.tile([128, 1], FP, tag="rall")
        nc.vector.tensor_reduce(
            rall[:nr], rmax[:nr], axis=AX.X, op=ALU.max)
        # sw = max(rmax/7, 1e-9)
        nc.vector.tensor_scalar(
            self.sw_pk[:nr, m:m + 1], rall[:nr], C7, 1e-9, ALU.mult, ALU.max)
        rw = ws_pool.tile([128, 1], FP, tag="rw")
        nc.vector.reciprocal(rw[:nr], self.sw_pk[:nr, m:m + 1])
        for hh in range(2):
            qst = qst_pool.tile([128, 2048], BF, tag="qst")
            nc.scalar.activation(
                halves[hh][:nr], halves[hh][:nr], AF.Identity,
                bias=self.magic_col[:nr], scale=rw[:nr])
            nc.scalar.activation(
                qst[:nr], halves[hh][:nr], AF.Identity,
                bias=self.neg_magic_col[:nr], scale=1.0)
            nc.sync.dma_start(
                self.qw_d[r0:r0 + nr, hh * 2048:(hh + 1) * 2048], qst[:nr])

    # ---------------- activation phase ----------------
    def act_chunk(self, c, xin_pool, xT_pool, st_pool, y_pool, xps_pool):
        nc = self.nc
        xT_lo = xT_pool.tile([128, KT // 2, CHUNK], FP, tag="xT")
        xT_hi = xT_pool.tile([128, KT // 2, CHUNK], FP, tag="xT")
        xT = [xT_lo, xT_hi]
        pcm = st_pool.tile([128, 2, KT], FP, tag="pcm")
        for th in range(2):
            xins = []
            for hh in range(2):
                xnat = xin_pool.tile([128, 2048], FP, tag="xn")
                nc.sync.dma_start(
                    xnat[:],
                    self.x_d[c * CHUNK + th * 128:c * CHUNK + (th + 1) * 128,
                             hh * 2048:(hh + 1) * 2048])
                xins.append(xnat)
            for kq in range(KT // 4):
                pst = xps_pool.tile([128, 512], FP, tag="pst")
                for j in range(4):
                    k = kq * 4 + j
                    nc.tensor.transpose(
                        pst[:, j * 128:(j + 1) * 128],
                        xins[k // 16][:, (k % 16) * 128:(k % 16 + 1) * 128],
                        self.ident[:])
                p3 = pst[:].rearrange("p (k t) -> p k t", k=4)
                nc.scalar.copy(
                    xT[kq // 4][:, (kq % 4) * 4:(kq % 4) * 4 + 4,
                                th * 128:(th + 1) * 128], p3)
                nc.vector.tensor_reduce(
                    pcm[:, th, kq * 4:kq * 4 + 4], p3, axis=AX.X, op=ALU.max,
                    apply_absolute_value=True)
        # ---- stats ----
        cmax = st_pool.tile([128, KT], FP, tag="cmax")
        nc.vector.scalar_tensor_tensor(
            cmax[:], pcm[:, 0, :], 0.0, pcm[:, 1, :],
            op0=ALU.bypass, op1=ALU.max)
        cmx1 = st_pool.tile([128, 1], FP, tag="cmx1")
        nc.vector.tensor_reduce(
            cmx1[:], cmax[:], axis=AX.X, op=ALU.max)
        tmax_b = st_pool.tile([128, 1], FP, tag="tmax_b")
        nc.gpsimd.partition_all_reduce(
            tmax_b[:], cmx1[:], 128, ReduceOp.max)
        nc.vector.tensor_scalar(
            self.m7_all[:, c:c + 1], tmax_b[:], C7, None, ALU.mult)
        # ---- pw = clamp(2^ceil(log2(cmax/tmax)), 2^-13, 1), exactly:
        # smallest power of two >= u via IEEE bit surgery ----
        rtm = st_pool.tile([128, 1], FP, tag="rtm")
        nc.vector.reciprocal(rtm[:], tmax_b[:])
        u = st_pool.tile([128, KT], FP, tag="u")
        nc.vector.tensor_scalar(u[:], cmax[:], rtm[:], None, ALU.mult)
        ui = u[:].bitcast(I32)
        tm = st_pool.tile([128, KT], I32, tag="tm")
        nc.vector.tensor_scalar(tm[:], ui, 0x007fffff, None, ALU.bitwise_and)
        # m + 0x7fffff has bit 23 set iff mantissa m != 0
        nc.vector.tensor_scalar(tm[:], tm[:], 0x007fffff, None, ALU.add)
        nc.vector.tensor_scalar(tm[:], tm[:], 0x00800000, None,
                                ALU.bitwise_and)
        te = st_pool.tile([128, KT], I32, tag="te")
        nc.vector.tensor_scalar(te[:], ui, 0x7f800000, None, ALU.bitwise_and)
        pw_t = st_pool.tile([128, KT], I32, tag="pw_t")
        nc.vector.scalar_tensor_tensor(
            pw_t[:], te[:], 0, tm[:], op0=ALU.bypass, op1=ALU.add)
        pw_f = st_pool.tile([128, KT], FP, tag="pw_f")
        nc.vector.tensor_scalar(
            pw_f[:], pw_t[:].bitcast(FP), 1.0, 2.0 ** -13, ALU.min, ALU.max)
        pw = pw_f[:]
        # quantize to the pw grid with a per-channel magic constant:
        #   t = x*(7/tmax) + MAGIC*pw   (rounds to multiples of pw, RNE)
        #   y = t - MAGIC*pw            (exact, Sterbenz)
        rt7 = st_pool.tile([128, 1], FP, tag="rt7")
        nc.vector.tensor_scalar(rt7[:], rtm[:], 7.0, None, ALU.mult)
        mgpw = st_pool.tile([128, KT], FP, tag="mgpw")
        nc.vector.tensor_scalar(mgpw[:], pw, MAGIC, None, ALU.mult)
        # ---- quantize: y = round(x*r) * pw  (fp8e5, exact) ----
        if c % 2 == 0:
            self.y_st = y_pool.tile([128, 2, KT, CHUNK], F8, tag="y_st")
        for half in range(2):
            X = xT[half][:]
            mg_bc = mgpw[:, half * 16:(half + 1) * 16, None].broadcast_to(
                [128, 16, CHUNK])
            nc.vector.scalar_tensor_tensor(
                X, X, rt7[:], mg_bc, op0=ALU.mult, op1=ALU.add)
            nc.vector.scalar_tensor_tensor(
                self.y_st[:, c % 2, half * 16:(half + 1) * 16, :],
                X, 0.0, mg_bc, op0=ALU.bypass, op1=ALU.subtract)
        if c % 2 == 1:
            nc.sync.dma_start(
                self.y_d[:, c - 1:c + 1, :, :], self.y_st[:])

    # ------------- qw sweep load (transpose + fp8 cast), spreadable ------
    def load_qw_part(self, s, k_lo, k_hi):
        nc = self.nc
        if s not in self.qw_sw:
            qw_sw = self.qw_pool.tile([128, KT, 1024], F8, tag="qw")
            self.qw_sw[s] = qw_sw
        qw_sw = self.qw_sw[s]
        for k in range(k_lo, k_hi):
            loc = 0
            for b in SWEEPS[s]:
                vo, w = VB_OFF[b], VB_W[b]
                land = self.land_pool.tile([128, 512], BF, tag="land")
                nc.sync.dma_start(
                    land[:, :w],
                    self.qw_d[vo:vo + w, k * 128:(k + 1) * 128],
                    transpose=True)
                self._copy(qw_sw[:, k, loc:loc + w], land[:, :w])
                loc += w

    # ---------------- matmul: one 256-token group of one sweep ----------
    def mm_grp(self, s, gc):
        nc = self.nc
        qw_sw = self.qw_sw[s]
        ba, bb = SWEEPS[s]
        wa, wb = VB_W[ba], VB_W[bb]
        vo = VB_OFF[ba]
        ygrp = self.ygrp_pool.tile([128, KT, CHUNK], F8, tag="yg")
        nc.sync.dma_start(ygrp[:], self.y_d[:, gc, :, :])
        m7c = self.m7_all[:, gc:gc + 1]
        for ti in range(2):
            tb = gc * 2 + ti
            ps_a = self.mps_pool.tile([128, 512], FP, tag="ps")
            ps_b = self.mps_pool.tile([128, 512], FP, tag="ps")
            for kp in range(KP):
                lhs = ygrp[:, 2 * kp:2 * kp + 2, ti * 128:(ti + 1) * 128]
                nc.tensor.matmul(
                    ps_a[:, :wa], lhs, qw_sw[:, 2 * kp:2 * kp + 2, 0:wa],
                    start=(kp == 0), stop=(kp == KP - 1), perf_mode=DR)
                nc.tensor.matmul(
                    ps_b[:, :wb], lhs,
                    qw_sw[:, 2 * kp:2 * kp + 2, 512:512 + wb],
                    start=(kp == 0), stop=(kp == KP - 1), perf_mode=DR)
            stg = self.stg_pool.tile([128, 1024], BF, tag="stg")
            self._alt2 ^= 1
            if self._alt2:
                nc.vector.tensor_scalar(
                    stg[:, 0:wa], ps_a[:, :wa], m7c, None, ALU.mult)
                nc.scalar.activation(
                    stg[:, wa:wa + wb], ps_b[:, :wb], AF.Copy, scale=m7c)
            else:
                nc.scalar.activation(
                    stg[:, 0:wa], ps_a[:, :wa], AF.Copy, scale=m7c)
                nc.vector.tensor_scalar(
                    stg[:, wa:wa + wb], ps_b[:, :wb], m7c, None, ALU.mult)
            nc.sync.dma_start(
                self.out_d[tb * 128:(tb + 1) * 128, vo:vo + wa + wb],
                stg[:, :wa + wb])


def _emit(ctx, tc, x_d, w_d, out_d):
    em = _Emitter(ctx, tc, x_d, w_d, out_d)
    nc = em.nc

    with ExitStack() as stage_a:
        cp2 = stage_a.enter_context(tc.tile_pool(name="consts2", bufs=1))
        em.neg_magic_col = cp2.tile([128, 1], FP)
        nc.vector.memset(em.neg_magic_col[:], -MAGIC)
        wq_pool = stage_a.enter_context(tc.tile_pool(name="wq", bufs=3))
        ws_pool = stage_a.enter_context(tc.tile_pool(name="wsm", bufs=2))
        qst_pool = stage_a.enter_context(tc.tile_pool(name="qst", bufs=2))
        xin_pool = stage_a.enter_context(tc.tile_pool(name="xin", bufs=2))
        xT_pool = stage_a.enter_context(tc.tile_pool(name="xT", bufs=3))
        st_pool = stage_a.enter_context(tc.tile_pool(name="xst", bufs=2))
        y_pool = stage_a.enter_context(tc.tile_pool(name="yst", bufs=1))
        xps_pool = stage_a.enter_context(
            tc.tile_pool(name="xps", bufs=2, space="PSUM"))

        # mm groups become issueable as their y chunk + qw sweep land.
        pending = deque()
        issued_chunks = [False] * NCHUNK
        loaded = [False] * 4

        def make_ready(c=None, s=None):
            if c is not None:
                issued_chunks[c] = True
            if s is not None:
                loaded[s] = True
            for sw in range(4):
                if loaded[sw]:
                    for gc in range(NCHUNK):
                        if issued_chunks[gc] and (sw, gc) not in scheduled:
                            scheduled.add((sw, gc))
                            pending.append((sw, gc))

        scheduled = set()

        def drain(n):
            for _ in range(min(n, len(pending))):
                s, gc = pending.popleft()
                em.mm_grp(s, gc)

        for i in range(32):
            em.weight_tile(i, wq_pool, ws_pool, qst_pool)
            if 8 <= i < 12:
                em.load_qw_part(0, (i - 8) * 8, (i - 7) * 8)
                if i == 11:
                    make_ready(s=0)
            if 16 <= i < 20:
                em.load_qw_part(1, (i - 16) * 8, (i - 15) * 8)
                if i == 19:
                    make_ready(s=1)
            if i % 2 == 1:
                c = (i - 1) // 2
                em.act_chunk(c, xin_pool, xT_pool, st_pool, y_pool, xps_pool)
                if c >= 1:
                    make_ready(c=c - 1)
                drain(2)
        make_ready(c=NCHUNK - 1)
        drain(len(pending))

    # sweeps 2 and 3: qw ring slots reused after sweeps 0/1 mm finished;
    # sweep 3's load is spread under sweep 2's matmuls
    em.load_qw_part(2, 0, KT)
    for s in (2, 3):
        for gc in range(NCHUNK):
            em.mm_grp(s, gc)
            if s == 2 and 4 <= gc < 12:
                em.load_qw_part(3, (gc - 4) * 4, (gc - 3) * 4)


_CACHED = None


def _build():
    global _CACHED
    if _CACHED is not None:
        return _CACHED
    nc = bacc.Bacc(
        "TRN2", target_bir_lowering=False, debug=False,
        enable_asserts=False, num_devices=NCORE)
    x_d = nc.dram_tensor("x", (T, H), FP, kind="ExternalInput").ap()
    w_d = nc.dram_tensor("w", (VSH, H), FP, kind="ExternalInput").ap()
    out_d = nc.dram_tensor("out", (T, VSH), BF, kind="ExternalOutput").ap()
    with tile.TileContext(nc) as tc:
        with ExitStack() as ctx:
            _emit(ctx, tc, x_d, w_d, out_d)
    nc.compile()
    _CACHED = nc
    return nc


def kernel(hidden_states: np.ndarray, lm_weight: np.ndarray) -> np.ndarray:
    b, t, h = hidden_states.shape
    assert (b * t, h) == (T, H) and lm_weight.shape == (V, H)
    x_full = np.ascontiguousarray(
        hidden_states.reshape(T, H).astype(np.float32))
    w_full = np.ascontiguousarray(lm_weight.astype(np.float32))
    in_maps = [{"x": x_full, "w": w_full[c * VSH:(c + 1) * VSH]}
               for c in range(NCORE)]
    nc = _build()
    res = run_bass_kernel_spmd(nc, in_maps, core_ids=list(range(NCORE)))
    # device output lacks the per-vocab-row sw scale; recompute it here
    # exactly as the device would (f32 rowmax * fl(1/7), floored at 1e-9)
    rowmax = np.abs(w_full).max(axis=1)
    sw = np.maximum(rowmax * np.float32(1.0 / 7.0), np.float32(1e-9))
    full = np.empty((T, V), dtype=np.float32)
    for c in range(NCORE):
        sl = slice(c * VSH, (c + 1) * VSH)
        full[:, sl] = np.asarray(res.results[c]["out"]).astype(np.float32)
        full[:, sl] *= sw[sl][None, :]
    return full.reshape(b, t, V)


# revision 26
# speedup vs baseline: 1.1106x; 1.0347x over previous
"""Trainium2 Bass kernel for the quantized LM-head (nn_LmHeadTender).

Math (per core, vocab-sharded, 4000 vocab rows/core):
  reference computes
    Wl   = dequant_int4(lm_weight)            # per-row scale sw = rowmax/7
    y    = dequant_int4(x, per-(chunk,channel) scale s = tmax*2^(bucket-13)/7)
    out  = y @ Wl.T
  Every scale is factored out of the matmul so both matmul operands are
  small integers (times powers of two) EXACTLY representable in fp8-e5m2
  (3 significand bits cover ints -8..7; exponent covers 2^-13..2^3):
    qw  in [-8, 7]                 (weight int values)
    yq  = qx * 2^(bucket-13)       (activation ints scaled by a power of 2)
    out[t, v] = (tmax_c/7) * sw[v] * sum_h yq[t, h] * qw[v, h]
  The fp8 matmul runs in DoubleRow perf mode (256-deep contraction per
  instruction, ~2x bf16 ALU throughput) and accumulates exactly in fp32
  PSUM. The device applies only the per-chunk tmax_c/7 scale; the
  per-vocab-row sw[v] (= max(rowmax/7, 1e-9), recomputable bitwise on the
  host from lm_weight alone) is applied by the host after the gather,
  which removes the on-device sw broadcast entirely.

  The reference clip to [-8,7] is dead code: |x/s| <= 7 and |w/sw| <= 7
  by construction of the scales, so quantization needs only the
  round-to-nearest-even magic-number trick.

Pipeline: weight tiles, activation chunks, and matmul token-groups are
interleaved in issue order so the PE matmul stream starts ~30us in and
everything else (DMA, DVE, ACT, GPSIMD) hides under it:
  - weight tile: DMA w f32, abs-rowmax (GPSIMD), 1/sw (DVE), quantize
    (ACT round pass + DVE/ACT sub pass) -> bf16 ints in DRAM qw_d.
  - act chunk: DMA x f32, PE-transpose (4 tiles per PSUM bank), partial
    per-channel abs-max directly on PSUM (DVE, pipelines with the
    copies), chunk stats (tmax/bucket/2^b via exact IEEE bit tricks),
    2-pass quantization alternated across DVE/ACT -> fp8e5 y in DRAM
    y_d [128, 32, T] (p-major: single 3D-AP DMAs).
  - mm: vocab in 4 sweeps of 2x512-col blocks (last 512+416); per sweep
    the qw block is DMA-transposed (bf16) + cast to fp8e5 once into a
    2-deep SBUF ring and reused by all 32 token blocks; stationary y tile
    shared by the 2 blocks; PSUM accumulates 16 k-pairs; epilogue just
    scales by tmax_c/7 (per-partition) -> bf16 out.
"""

import numpy as np
from collections import deque
from contextlib import ExitStack

import concourse.bass as bass
import concourse.tile as tile
from concourse import bacc, masks, mybir
from concourse.bass_isa import ReduceOp
from concourse.bass_utils import run_bass_kernel_spmd

FP = mybir.dt.float32
BF = mybir.dt.bfloat16
F8 = mybir.dt.float8e5
I32 = mybir.dt.int32
ALU = mybir.AluOpType
AX = mybir.AxisListType
AF = mybir.ActivationFunctionType
DR = mybir.MatmulPerfMode.DoubleRow

T = 4096            # tokens (2*2048)
H = 4096            # hidden
V = 32000           # vocab
NCORE = 8
VSH = V // NCORE    # 4000 vocab rows per core
CHUNK = 256
NCHUNK = T // CHUNK  # 16
DECOMP = 14
QMAX = 7.0
MAGIC = 12582912.0   # 1.5 * 2^23: round-to-nearest-even via add/sub
C7 = float(np.float32(1.0) / np.float32(7.0))  # fl(1/7); no divide op

KT = H // 128        # 32 k tiles
VT = 32              # weight row tiles (31 full + 1 of 32 rows)
KP = KT // 2         # 16 k pairs for DoubleRow
TB = T // 128        # 32 token blocks

VB_OFF = [0, 512, 1024, 1536, 2048, 2560, 3072, 3584]
VB_W = [512] * 7 + [416]
SWEEPS = [(0, 1), (2, 3), (4, 5), (6, 7)]


def _vt_rows(m):
    r0 = m * 128
    return r0, min(r0 + 128, VSH) - r0


class _Emitter:
    def __init__(self, ctx, tc, x_d, w_d, out_d):
        self.tc = tc
        self.nc = tc.nc
        self.x_d = x_d
        self.w_d = w_d
        self.out_d = out_d
        nc = self.nc

        cpool = ctx.enter_context(tc.tile_pool(name="consts", bufs=1))
        self.ident = cpool.tile([128, 128], FP)
        masks.make_identity(nc, self.ident[:])
        self.magic_col = cpool.tile([128, 1], FP)
        nc.vector.memset(self.magic_col[:], MAGIC)
        self.sw_pk = cpool.tile([128, 32], FP)    # sw packed; v = m*128+p
        nc.vector.memset(self.sw_pk[:], 0.0)
        self.m7_all = cpool.tile([128, 16], FP)   # tmax_c/7 per chunk

        dpool = ctx.enter_context(tc.tile_pool(name="dram", bufs=1,
                                               space="DRAM"))
        self.qw_d = dpool.tile([VSH, H], BF)      # weight ints, [v, h]
        self.y_d = dpool.tile([128, NCHUNK, KT, CHUNK], F8)  # chunk-major

        self.mps_pool = ctx.enter_context(
            tc.tile_pool(name="mps", bufs=5, space="PSUM"))
        self.stg_pool = ctx.enter_context(tc.tile_pool(name="stg", bufs=2))
        self.ygrp_pool = ctx.enter_context(tc.tile_pool(name="ygrp", bufs=2))
        self.land_pool = ctx.enter_context(tc.tile_pool(name="land", bufs=2))
        self.qw_pool = ctx.enter_context(tc.tile_pool(name="qw", bufs=2))
        self._alt = 0
        self._alt2 = 0
        self._alt3 = 0
        self.qw_sw = {}

    def _copy(self, dst, src):
        self._alt ^= 1
        if self._alt:
            self.nc.vector.tensor_copy(dst, src)
        else:
            self.nc.scalar.copy(dst, src)

    def _copy3(self, dst, src):
        self._alt3 = (self._alt3 + 1) % 3
        if self._alt3 == 0:
            self.nc.gpsimd.tensor_copy(dst, src)
        elif self._alt3 == 1:
            self.nc.vector.tensor_copy(dst, src)
        else:
            self.nc.scalar.copy(dst, src)

    # ---------------- weight phase ----------------
    def weight_tile(self, m, wq_pool, ws_pool, qst_pool):
        nc = self.nc
        r0, nr = _vt_rows(m)
        halves = []
        for hh in range(2):
            w_nat = wq_pool.tile([128, 2048], FP, tag="w_nat")
            nc.sync.dma_start(
                w_nat[:nr], self.w_d[r0:r0 + nr, hh * 2048:(hh + 1) * 2048])
            halves.append(w_nat)
        rmax = ws_pool.tile([128, 2], FP, tag="rmax")
        for hh in range(2):
            nc.vector.tensor_reduce(
                rmax[:nr, hh:hh + 1], halves[hh][:nr], axis=AX.X, op=ALU.max,
                apply_absolute_value=True)
        rall = ws_pool.tile([128, 1], FP, tag="rall")
        nc.vector.tensor_reduce(
            rall[:nr], rmax[:nr], axis=AX.X, op=ALU.max)
        # sw = max(rmax/7, 1e-9)
        nc.vector.tensor_scalar(
            self.sw_pk[:nr, m:m + 1], rall[:nr], C7, 1e-9, ALU.mult, ALU.max)
        rw = ws_pool.tile([128, 1], FP, tag="rw")
        nc.vector.reciprocal(rw[:nr], self.sw_pk[:nr, m:m + 1])
        for hh in range(2):
            qst = qst_pool.tile([128, 2048], BF, tag="qst")
            nc.scalar.activation(
                halves[hh][:nr], halves[hh][:nr], AF.Identity,
                bias=self.magic_col[:nr], scale=rw[:nr])
            nc.scalar.activation(
                qst[:nr], halves[hh][:nr], AF.Identity,
                bias=self.neg_magic_col[:nr], scale=1.0)
            nc.sync.dma_start(
                self.qw_d[r0:r0 + nr, hh * 2048:(hh + 1) * 2048], qst[:nr])

    # ---------------- activation phase ----------------
    def act_chunk(self, c, xin_pool, xT_pool, st_pool, y_pool, xps_pool):
        nc = self.nc
        xT_lo = xT_pool.tile([128, KT // 2, CHUNK], FP, tag="xT")
        xT_hi = xT_pool.tile([128, KT // 2, CHUNK], FP, tag="xT")
        xT = [xT_lo, xT_hi]
        for th in range(2):
            xins = []
            for hh in range(2):
                xnat = xin_pool.tile([128, 2048], FP, tag="xn")
                nc.sync.dma_start(
                    xnat[:],
                    self.x_d[c * CHUNK + th * 128:c * CHUNK + (th + 1) * 128,
                             hh * 2048:(hh + 1) * 2048])
                xins.append(xnat)
            for kq in range(KT // 4):
                pst = xps_pool.tile([128, 512], FP, tag="pst")
                for j in range(4):
                    k = kq * 4 + j
                    nc.tensor.transpose(
                        pst[:, j * 128:(j + 1) * 128],
                        xins[k // 16][:, (k % 16) * 128:(k % 16 + 1) * 128],
                        self.ident[:])
                p3 = pst[:].rearrange("p (k t) -> p k t", k=4)
                self._copy(
                    xT[kq // 4][:, (kq % 4) * 4:(kq % 4) * 4 + 4,
                                th * 128:(th + 1) * 128], p3)
        # ---- stats ----
        cmax = st_pool.tile([128, KT], FP, tag="cmax")
        for half in range(2):
            nc.vector.tensor_reduce(
                cmax[:, half * 16:(half + 1) * 16], xT[half][:], axis=AX.X,
                op=ALU.max, apply_absolute_value=True)
        cmx1 = st_pool.tile([128, 1], FP, tag="cmx1")
        nc.vector.tensor_reduce(
            cmx1[:], cmax[:], axis=AX.X, op=ALU.max)
        tmax_b = st_pool.tile([128, 1], FP, tag="tmax_b")
        nc.gpsimd.partition_all_reduce(
            tmax_b[:], cmx1[:], 128, ReduceOp.max)
        nc.vector.tensor_scalar(
            self.m7_all[:, c:c + 1], tmax_b[:], C7, None, ALU.mult)
        # ---- pw = clamp(2^ceil(log2(cmax/tmax)), 2^-13, 1), exactly:
        # smallest power of two >= u via IEEE bit surgery ----
        rtm = st_pool.tile([128, 1], FP, tag="rtm")
        nc.vector.reciprocal(rtm[:], tmax_b[:])
        u = st_pool.tile([128, KT], FP, tag="u")
        nc.vector.tensor_scalar(u[:], cmax[:], rtm[:], None, ALU.mult)
        ui = u[:].bitcast(I32)
        tm = st_pool.tile([128, KT], I32, tag="tm")
        nc.vector.tensor_scalar(tm[:], ui, 0x007fffff, None, ALU.bitwise_and)
        # m + 0x7fffff has bit 23 set iff mantissa m != 0
        nc.vector.tensor_scalar(tm[:], tm[:], 0x007fffff, None, ALU.add)
        nc.vector.tensor_scalar(tm[:], tm[:], 0x00800000, None,
                                ALU.bitwise_and)
        te = st_pool.tile([128, KT], I32, tag="te")
        nc.vector.tensor_scalar(te[:], ui, 0x7f800000, None, ALU.bitwise_and)
        pw_t = st_pool.tile([128, KT], I32, tag="pw_t")
        nc.vector.scalar_tensor_tensor(
            pw_t[:], te[:], 0, tm[:], op0=ALU.bypass, op1=ALU.add)
        pw_f = st_pool.tile([128, KT], FP, tag="pw_f")
        nc.vector.tensor_scalar(
            pw_f[:], pw_t[:].bitcast(FP), 1.0, 2.0 ** -13, ALU.min, ALU.max)
        pw = pw_f[:]
        # quantize to the pw grid with a per-channel magic constant:
        #   t = x*(7/tmax) + MAGIC*pw   (rounds to multiples of pw, RNE)
        #   y = t - MAGIC*pw            (exact, Sterbenz)
        rt7 = st_pool.tile([128, 1], FP, tag="rt7")
        nc.vector.tensor_scalar(rt7[:], rtm[:], 7.0, None, ALU.mult)
        mgpw = st_pool.tile([128, KT], FP, tag="mgpw")
        nc.vector.tensor_scalar(mgpw[:], pw, MAGIC, None, ALU.mult)
        # ---- quantize: y = round(x*r) * pw  (fp8e5, exact) ----
        if c % 2 == 0:
            self.y_st = y_pool.tile([128, 2, KT, CHUNK], F8, tag="y_st")
        for half in range(2):
            X = xT[half][:]
            mg_bc = mgpw[:, half * 16:(half + 1) * 16, None].broadcast_to(
                [128, 16, CHUNK])
            nc.vector.scalar_tensor_tensor(
                X, X, rt7[:], mg_bc, op0=ALU.mult, op1=ALU.add)
            nc.vector.scalar_tensor_tensor(
                self.y_st[:, c % 2, half * 16:(half + 1) * 16, :],
                X, 0.0, mg_bc, op0=ALU.bypass, op1=ALU.subtract)
        if c % 2 == 1:
            nc.sync.dma_start(
                self.y_d[:, c - 1:c + 1, :, :], self.y_st[:])

    # ------------- qw sweep load (transpose + fp8 cast), spreadable ------
    def load_qw_part(self, s, k_lo, k_hi):
        nc = self.nc
        if s not in self.qw_sw:
            qw_sw = self.qw_pool.tile([128, KT, 1024], F8, tag="qw")
            self.qw_sw[s] = qw_sw
        qw_sw = self.qw_sw[s]
        for k in range(k_lo, k_hi):
            loc = 0
            for b in SWEEPS[s]:
                vo, w = VB_OFF[b], VB_W[b]
                land = self.land_pool.tile([128, 512], BF, tag="land")
                nc.sync.dma_start(
                    land[:, :w],
                    self.qw_d[vo:vo + w, k * 128:(k + 1) * 128],
                    transpose=True)
                self._copy(qw_sw[:, k, loc:loc + w], land[:, :w])
                loc += w

    # ---------------- matmul: one 256-token group of one sweep ----------
    def mm_grp(self, s, gc):
        nc = self.nc
        qw_sw = self.qw_sw[s]
        ba, bb = SWEEPS[s]
        wa, wb = VB_W[ba], VB_W[bb]
        vo = VB_OFF[ba]
        ygrp = self.ygrp_pool.tile([128, KT, CHUNK], F8, tag="yg")
        nc.sync.dma_start(ygrp[:], self.y_d[:, gc, :, :])
        m7c = self.m7_all[:, gc:gc + 1]
        for ti in range(2):
            tb = gc * 2 + ti
            ps_a = self.mps_pool.tile([128, 512], FP, tag="ps")
            ps_b = self.mps_pool.tile([128, 512], FP, tag="ps")
            for kp in range(KP):
                lhs = ygrp[:, 2 * kp:2 * kp + 2, ti * 128:(ti + 1) * 128]
                nc.tensor.matmul(
                    ps_a[:, :wa], lhs, qw_sw[:, 2 * kp:2 * kp + 2, 0:wa],
                    start=(kp == 0), stop=(kp == KP - 1), perf_mode=DR)
                nc.tensor.matmul(
                    ps_b[:, :wb], lhs,
                    qw_sw[:, 2 * kp:2 * kp + 2, 512:512 + wb],
                    start=(kp == 0), stop=(kp == KP - 1), perf_mode=DR)
            stg = self.stg_pool.tile([128, 1024], BF, tag="stg")
            self._alt2 ^= 1
            if self._alt2:
                nc.vector.tensor_scalar(
                    stg[:, 0:wa], ps_a[:, :wa], m7c, None, ALU.mult)
                nc.scalar.activation(
                    stg[:, wa:wa + wb], ps_b[:, :wb], AF.Copy, scale=m7c)
            else:
                nc.scalar.activation(
                    stg[:, 0:wa], ps_a[:, :wa], AF.Copy, scale=m7c)
                nc.vector.tensor_scalar(
                    stg[:, wa:wa + wb], ps_b[:, :wb], m7c, None, ALU.mult)
            nc.sync.dma_start(
                self.out_d[tb * 128:(tb + 1) * 128, vo:vo + wa + wb],
                stg[:, :wa + wb])


def _emit(ctx, tc, x_d, w_d, out_d):
    em = _Emitter(ctx, tc, x_d, w_d, out_d)
    nc = em.nc

    with ExitStack() as stage_a:
        cp2 = stage_a.enter_context(tc.tile_pool(name="consts2", bufs=1))
        em.neg_magic_col = cp2.tile([128, 1], FP)
        nc.vector.memset(em.neg_magic_col[:], -MAGIC)
        wq_pool = stage_a.enter_context(tc.tile_pool(name="wq", bufs=3))
        ws_pool = stage_a.enter_context(tc.tile_pool(name="wsm", bufs=2))
        qst_pool = stage_a.enter_context(tc.tile_pool(name="qst", bufs=2))
        xin_pool = stage_a.enter_context(tc.tile_pool(name="xin", bufs=2))
        xT_pool = stage_a.enter_context(tc.tile_pool(name="xT", bufs=3))
        st_pool = stage_a.enter_context(tc.tile_pool(name="xst", bufs=2))
        y_pool = stage_a.enter_context(tc.tile_pool(name="yst", bufs=1))
        xps_pool = stage_a.enter_context(
            tc.tile_pool(name="xps", bufs=3, space="PSUM"))

        # mm groups become issueable as their y chunk + qw sweep land.
        pending = deque()
        issued_chunks = [False] * NCHUNK
        loaded = [False] * 4

        def make_ready(c=None, s=None):
            if c is not None:
                issued_chunks[c] = True
            if s is not None:
                loaded[s] = True
            for sw in range(4):
                if loaded[sw]:
                    for gc in range(NCHUNK):
                        if issued_chunks[gc] and (sw, gc) not in scheduled:
                            scheduled.add((sw, gc))
                            pending.append((sw, gc))

        scheduled = set()

        def drain(n):
            for _ in range(min(n, len(pending))):
                s, gc = pending.popleft()
                em.mm_grp(s, gc)

        for i in range(32):
            em.weight_tile(i, wq_pool, ws_pool, qst_pool)
            if 8 <= i < 12:
                em.load_qw_part(0, (i - 8) * 8, (i - 7) * 8)
                if i == 11:
                    make_ready(s=0)
            if 16 <= i < 20:
                em.load_qw_part(1, (i - 16) * 8, (i - 15) * 8)
                if i == 19:
                    make_ready(s=1)
            if i % 2 == 1:
                c = (i - 1) // 2
                em.act_chunk(c, xin_pool, xT_pool, st_pool, y_pool, xps_pool)
                if c >= 1:
                    make_ready(c=c - 1)
                drain(2)
        make_ready(c=NCHUNK - 1)
        drain(len(pending))

    # sweeps 2 and 3: qw ring slots reused after sweeps 0/1 mm finished;
    # sweep 3's load is spread under sweep 2's matmuls
    em.load_qw_part(2, 0, KT)
    for s in (2, 3):
        for gc in range(NCHUNK):
            em.mm_grp(s, gc)
            if s == 2 and 4 <= gc < 12:
                em.load_qw_part(3, (gc - 4) * 4, (gc - 3) * 4)


_CACHED = None


def _build():
    global _CACHED
    if _CACHED is not None:
        return _CACHED
    nc = bacc.Bacc(
        "TRN2", target_bir_lowering=False, debug=False,
        enable_asserts=False, num_devices=NCORE)
    x_d = nc.dram_tensor("x", (T, H), FP, kind="ExternalInput").ap()
    w_d = nc.dram_tensor("w", (VSH, H), FP, kind="ExternalInput").ap()
    out_d = nc.dram_tensor("out", (T, VSH), BF, kind="ExternalOutput").ap()
    with tile.TileContext(nc) as tc:
        with ExitStack() as ctx:
            _emit(ctx, tc, x_d, w_d, out_d)
    nc.compile()
    _CACHED = nc
    return nc


def kernel(hidden_states: np.ndarray, lm_weight: np.ndarray) -> np.ndarray:
    b, t, h = hidden_states.shape
    assert (b * t, h) == (T, H) and lm_weight.shape == (V, H)
    x_full = np.ascontiguousarray(
        hidden_states.reshape(T, H).astype(np.float32))
    w_full = np.ascontiguousarray(lm_weight.astype(np.float32))
    in_maps = [{"x": x_full, "w": w_full[c * VSH:(c + 1) * VSH]}
               for c in range(NCORE)]
    nc = _build()
    res = run_bass_kernel_spmd(nc, in_maps, core_ids=list(range(NCORE)))
    # device output lacks the per-vocab-row sw scale; recompute it here
    # exactly as the device would (f32 rowmax * fl(1/7), floored at 1e-9)
    rowmax = np.abs(w_full).max(axis=1)
    sw = np.maximum(rowmax * np.float32(1.0 / 7.0), np.float32(1e-9))
    full = np.empty((T, V), dtype=np.float32)
    for c in range(NCORE):
        sl = slice(c * VSH, (c + 1) * VSH)
        full[:, sl] = np.asarray(res.results[c]["out"]).astype(np.float32)
        full[:, sl] *= sw[sl][None, :]
    return full.reshape(b, t, V)
